# revision 9
# baseline (speedup 1.0000x reference)
"""Trainium2 Bass kernel for nn_ConvDiscriminator (ragged CNN discriminator).

Math (per sample b with length L):
  flat = encoder_output[0:L, b, :].ravel()           # contiguous [L*512]
  X[h, l] = flat[h*L + l]  (raw reshape to [512, L], zero-pad cols >= L)
  conv_w (w=1..5): out_w[f, t] = sum_{h,dw} Ww[f,h,dw] * X[h, t+dw]
  pool_w[f] = relu(bias_w[f] + max_{t <= Leff-w} out_w[f, t])
  fc1 -> fc2 -> sigmoid

Kernel strategy (8 cores, uniform SPMD program, per-core data tables):
  - Sort the 128 samples by length desc; slot j holds ranks [8j, 8j+8), one
    per core.  Canonical slot width Wc[j] = max length in slot (baked into
    the program); each core's actual lengths live only in data (offset
    tables + masks), so one program serves all 8 cores.
  - Per sample, one indirect (gather) DMA builds SBUF tile
    F[p, k*Wc + j] = flat[(4p+k)*L + j] from 512 host-computed chunk
    offsets.  Matmul k-tile k of the contraction over h = rows h' = 4p+k,
    with conv weights pre-permuted on host to match.
  - conv w outputs accumulate in one PSUM bank per (slot, w): 4*w matmuls
    (dw-shift folded into the rhs column window) plus one K=1 matmul that
    adds -1e30 to invalid output columns (mask rows are host data), so
    pool = relu(bias + reduce_max(psum)).
  - Tiny fc1/fc2/sigmoid on-chip, output [1, 16] per core.
"""

import os
import sys

for _p in ("/opt/trn_rl_repo", "/root/.axon_site/_ro/trn_rl_repo"):
    if os.path.isdir(_p) and _p not in sys.path:
        sys.path.insert(0, _p)

import numpy as np
import ml_dtypes

T = 512
B = 128
H = 512
NF = 128
FS = 5
P = 128
NCORES = 8
NSLOT = B // NCORES  # 16
SAMP = T * H  # elements per sample block
MW = 520  # per-slot mask row width

USE_BF16 = True

LAST_EXEC_NS = None
LAST_RESULTS = None
_PROGRAM_CACHE = {}


def _pair_index(w, dw):
    # enumerate (w, dw) pairs: w=1..5, dw=0..w-1 -> 0..14
    return (w - 1) * w // 2 + dw


def build_program(Wc, use_bf16=True):
    import concourse.bass as bass
    import concourse.bacc as bacc
    import concourse.mybir as mybir
    from concourse.tile import TileContext

    f32 = mybir.dt.float32
    cdt = mybir.dt.bfloat16 if use_bf16 else f32
    i32 = mybir.dt.int32
    AX = mybir.AxisListType
    AF = mybir.ActivationFunctionType

    nc = bacc.Bacc()
    enc = nc.declare_dram_parameter("enc", [NSLOT * SAMP, 1], f32, isOutput=False)
    idx = nc.declare_dram_parameter("idx", [P, NSLOT * 4], i32, isOutput=False)
    msk = nc.declare_dram_parameter("msk", [1, NSLOT * MW], cdt, isOutput=False)
    neg = nc.declare_dram_parameter("neg", [1, P], cdt, isOutput=False)
    wconv = nc.declare_dram_parameter("wconv", [P, 60 * P], cdt, isOutput=False)
    cbias = nc.declare_dram_parameter("cbias", [P, FS], f32, isOutput=False)
    fc1w = nc.declare_dram_parameter("fc1w", [P, 5 * 100], cdt, isOutput=False)
    fc1b = nc.declare_dram_parameter("fc1b", [100, 1], f32, isOutput=False)
    fc2w = nc.declare_dram_parameter("fc2w", [100, 1], cdt, isOutput=False)
    fc2b = nc.declare_dram_parameter("fc2b", [1, 1], f32, isOutput=False)
    out = nc.declare_dram_parameter("out", [1, NSLOT], f32, isOutput=True)

    # process slots shortest-first so the PE starts quickly
    order = sorted(range(NSLOT), key=lambda j: Wc[j])
    groups = [order[i : i + 4] for i in range(0, NSLOT, 4)]

    with TileContext(nc) as tc:
        with (
            tc.tile_pool(name="const", bufs=1) as constp,
            tc.tile_pool(name="fpool", bufs=16) as fpool,
            tc.tile_pool(name="pspool", bufs=8, space="PSUM") as pspool,
        ):
            wsb = constp.tile([P, 60 * P], cdt, tag="wsb")
            nc.sync.dma_start(out=wsb[:], in_=wconv[:])
            idx_sb = constp.tile([P, NSLOT * 4], i32, tag="idx")
            nc.sync.dma_start(out=idx_sb[:], in_=idx[:])
            msk_sb = constp.tile([1, NSLOT * MW], cdt, tag="msk")
            nc.sync.dma_start(out=msk_sb[:], in_=msk[:])
            neg_sb = constp.tile([1, P], cdt, tag="neg")
            nc.sync.dma_start(out=neg_sb[:], in_=neg[:])
            cb_sb = constp.tile([P, FS], f32, tag="cb")
            nc.sync.dma_start(out=cb_sb[:], in_=cbias[:])
            fc1w_sb = constp.tile([P, 5 * 100], cdt, tag="fc1w")
            nc.sync.dma_start(out=fc1w_sb[:], in_=fc1w[:])
            fc1b_sb = constp.tile([100, 1], f32, tag="fc1b")
            nc.sync.dma_start(out=fc1b_sb[:], in_=fc1b[:])
            fc2w_sb = constp.tile([100, 1], cdt, tag="fc2w")
            nc.sync.dma_start(out=fc2w_sb[:], in_=fc2w[:])
            fc2b_sb = constp.tile([1, 1], f32, tag="fc2b")
            nc.sync.dma_start(out=fc2b_sb[:], in_=fc2b[:])

            # pool results: pools[w-1] fp32 [128, NSLOT]; bf16 relu'd copies
            pools = []
            poolsr = []
            for w in range(1, FS + 1):
                pw = constp.tile([P, NSLOT], f32, tag=f"pool{w}", name=f"pool{w}")
                pr = constp.tile([P, NSLOT], cdt, tag=f"poolr{w}", name=f"poolr{w}")
                pools.append(pw)
                poolsr.append(pr)

            fts = {}
            for gi, grp in enumerate(groups):
                for j in grp:
                    ft = fpool.tile([P, 4 * Wc[j]], cdt, tag="F", name=f"ft{j}")
                    # HW indirect DMA consumes one index per dest partition row
                    for k in range(4):
                        nc.gpsimd.indirect_dma_start(
                            out=ft[:, k * Wc[j] : (k + 1) * Wc[j]],
                            out_offset=None,
                            in_=enc[:],
                            in_offset=bass.IndirectOffsetOnAxis(
                                ap=idx_sb[:, j * 4 + k : j * 4 + k + 1], axis=0
                            ),
                        )
                    fts[j] = ft
                for w in range(1, FS + 1):
                    pss = {}
                    for j in grp:
                        Nw = Wc[j] - w + 1
                        ps = pspool.tile([P, Nw], f32, tag="ps", name=f"ps{j}w{w}")
                        # mask matmul: psum[f, t] += -1e30 * M[j*MW + w + t]
                        nc.tensor.matmul(
                            ps[:],
                            neg_sb[:],
                            msk_sb[:, j * MW + w : j * MW + w + Nw],
                            start=True,
                            stop=False,
                        )
                        pss[j] = ps
                    for dw in range(w):
                        i = _pair_index(w, dw)
                        for k in range(4):
                            last = (dw == w - 1) and (k == 3)
                            for j in grp:
                                Nw = Wc[j] - w + 1
                                c0 = k * Wc[j] + dw
                                nc.tensor.matmul(
                                    pss[j][:],
                                    wsb[:, (i * 4 + k) * P : (i * 4 + k + 1) * P],
                                    fts[j][:, c0 : c0 + Nw],
                                    start=False,
                                    stop=last,
                                )
                    for j in grp:
                        nc.vector.reduce_max(
                            pools[w - 1][:, j : j + 1], pss[j][:], axis=AX.X
                        )

            # pool_w = relu(max + bias_w)
            for w in range(1, FS + 1):
                nc.scalar.activation(
                    poolsr[w - 1][:],
                    pools[w - 1][:],
                    AF.Relu,
                    bias=cb_sb[:, w - 1 : w],
                )

            psf1 = pspool.tile([100, NSLOT], f32, tag="ps", name="psf1")
            for k in range(5):
                nc.tensor.matmul(
                    psf1[:],
                    fc1w_sb[:, k * 100 : (k + 1) * 100],
                    poolsr[k][:],
                    start=(k == 0),
                    stop=(k == 4),
                )
            fc1_sb = constp.tile([100, NSLOT], cdt, tag="fc1o")
            nc.scalar.activation(fc1_sb[:], psf1[:], AF.Identity, bias=fc1b_sb[:])

            psf2 = pspool.tile([1, NSLOT], f32, tag="ps", name="psf2")
            nc.tensor.matmul(psf2[:], fc2w_sb[:], fc1_sb[:], start=True, stop=True)
            out_sb = constp.tile([1, NSLOT], f32, tag="outsb")
            nc.scalar.activation(out_sb[:], psf2[:], AF.Sigmoid, bias=fc2b_sb[:])
            nc.sync.dma_start(out=out[:], in_=out_sb[:])

    nc.compile()
    return nc


def prepare(encoder_output, lengths, conv_ws, conv_bs, fc1_w, fc1_b, fc2_w, fc2_b,
            use_bf16=None):
    """Host-side prep: sample assignment, per-core data tables, program build.

    Returns (nc, in_maps, assignment) where assignment[c][j] = global sample.
    """
    if use_bf16 is None:
        use_bf16 = USE_BF16
    enc = np.ascontiguousarray(np.asarray(encoder_output, dtype=np.float32))
    lens = np.asarray(lengths).astype(np.int64)
    assert enc.shape == (T, B, H)
    assert lens.shape == (B,)

    cdt = ml_dtypes.bfloat16 if use_bf16 else np.float32

    # effective lengths (L < FS samples get rebuilt blocks with L_eff = FS)
    eff = np.maximum(lens, FS)

    # sort desc by effective length; slot j <- ranks [8j, 8j+8)
    ranks = np.argsort(-eff, kind="stable")
    assignment = [[int(ranks[8 * j + c]) for j in range(NSLOT)] for c in range(NCORES)]
    Wc = tuple(int(eff[ranks[8 * j]]) for j in range(NSLOT))

    encT = enc.transpose(1, 0, 2)  # [B, T, H], sample-major views

    in_maps = []
    for c in range(NCORES):
        enc_c = np.empty((NSLOT, T, H), dtype=np.float32)
        idx_c = np.empty((P, NSLOT * 4), dtype=np.int32)
        msk_c = np.zeros((1, NSLOT * MW), dtype=np.float32)
        for j in range(NSLOT):
            b = assignment[c][j]
            L = int(lens[b])
            Le = int(eff[b])
            if L >= FS:
                enc_c[j] = encT[b]
            else:
                # rebuild: flat'[h*FS + jj] = flat[h*L + jj] for jj < L else 0
                blk = np.zeros((T, H), dtype=np.float32)
                flat = encT[b].reshape(-1)[: H * L]
                v = np.zeros((H, FS), dtype=np.float32)
                v[:, :L] = flat.reshape(H, L)
                blk.reshape(-1)[: H * FS] = v.reshape(-1)
                enc_c[j] = blk
            base = j * SAMP
            pk = np.arange(P)[:, None] * 4 + np.arange(4)[None, :]  # [128, 4]
            idx_c[:, j * 4 : (j + 1) * 4] = base + pk * Le
            u = np.arange(MW)
            msk_c[0, j * MW : (j + 1) * MW] = (u > Le).astype(np.float32)

        in_maps.append(
            {
                "enc": enc_c.reshape(NSLOT * SAMP, 1),
                "idx": idx_c,
                "msk": msk_c.astype(cdt),
                "neg": np.full((1, P), -1e30, dtype=cdt),
            }
        )

    # weights, shared across cores
    wconv = np.empty((P, 60 * P), dtype=np.float32)
    hsel = np.arange(P)[:, None] * 4  # [128,1]
    for w in range(1, FS + 1):
        Ww = np.asarray(conv_ws[w - 1], dtype=np.float32)  # [NF, 1, H, w]
        for dw in range(w):
            i = _pair_index(w, dw)
            for k in range(4):
                # lhsT[p, f] = Ww[f, 0, 4p+k, dw]
                wconv[:, (i * 4 + k) * P : (i * 4 + k + 1) * P] = Ww[
                    :, 0, (hsel + k).ravel(), dw
                ].T
    cbias = np.stack([np.asarray(b, dtype=np.float32) for b in conv_bs], axis=1)
    fc1w_host = np.empty((P, 5 * 100), dtype=np.float32)
    fc1_w = np.asarray(fc1_w, dtype=np.float32)  # [100, 640]
    for k in range(5):
        fc1w_host[:, k * 100 : (k + 1) * 100] = fc1_w[:, k * P : (k + 1) * P].T
    shared = {
        "wconv": wconv.astype(cdt),
        "cbias": cbias,
        "fc1w": fc1w_host.astype(cdt),
        "fc1b": np.asarray(fc1_b, dtype=np.float32).reshape(100, 1),
        "fc2w": np.asarray(fc2_w, dtype=np.float32).T.astype(cdt).reshape(100, 1),
        "fc2b": np.asarray(fc2_b, dtype=np.float32).reshape(1, 1),
    }
    for m in in_maps:
        m.update(shared)

    key = (Wc, use_bf16)
    if key not in _PROGRAM_CACHE:
        _PROGRAM_CACHE[key] = build_program(Wc, use_bf16)
    nc = _PROGRAM_CACHE[key]
    return nc, in_maps, assignment


def _ensure_ntff_hook():
    """Install the axon NTFF profile hook if the image's antenv lacks it."""
    import types

    try:
        from antenv.axon_hooks import get_axon_ntff_profile_hook  # noqa: F401
        return True
    except ImportError:
        pass
    try:
        import antenv
        from trn_agent_boot.trn_boot import _ntff_profile_via_ctypes

        hook = _ntff_profile_via_ctypes("/opt/axon/libaxon_pjrt.so")
        mod = types.ModuleType("antenv.axon_hooks")
        _state = {"hook": hook}
        mod.get_axon_ntff_profile_hook = lambda: _state["hook"]
        mod.set_axon_ntff_profile_hook = lambda h: _state.update(hook=h)
        sys.modules["antenv.axon_hooks"] = mod
        antenv.axon_hooks = mod
        return hook is not None
    except Exception as e:  # pragma: no cover
        print(f"ntff hook install failed: {e}", file=sys.stderr)
        return False


def kernel(encoder_output, lengths,
           conv_w1, conv_b1, conv_w2, conv_b2, conv_w3, conv_b3,
           conv_w4, conv_b4, conv_w5, conv_b5,
           fc1_w, fc1_b, fc2_w, fc2_b):
    global LAST_EXEC_NS, LAST_RESULTS
    from concourse.bass_utils import run_bass_kernel_spmd

    conv_ws = [conv_w1, conv_w2, conv_w3, conv_w4, conv_w5]
    conv_bs = [conv_b1, conv_b2, conv_b3, conv_b4, conv_b5]
    nc, in_maps, assignment = prepare(
        encoder_output, lengths, conv_ws, conv_bs, fc1_w, fc1_b, fc2_w, fc2_b
    )

    trace = bool(int(os.environ.get("KERNEL_TRACE", "0")))
    if trace:
        trace = _ensure_ntff_hook()
    res = run_bass_kernel_spmd(nc, in_maps, list(range(NCORES)), trace=trace)
    LAST_RESULTS = res
    LAST_EXEC_NS = getattr(res, "exec_time_ns", None)

    out_full = np.empty((B, 1, 1), dtype=np.float32)
    for c in range(NCORES):
        oc = np.asarray(res.results[c]["out"]).reshape(NSLOT)
        for j in range(NSLOT):
            out_full[assignment[c][j], 0, 0] = oc[j]
    return out_full


# revision 15
# speedup vs baseline: 1.1290x; 1.1290x over previous
"""Trainium2 Bass kernel for nn_ConvDiscriminator (ragged CNN discriminator).

Math (per sample b with length L):
  flat = encoder_output[0:L, b, :].ravel()           # contiguous [L*512]
  X[h, l] = flat[h*L + l]  (raw reshape to [512, L], zero-pad cols >= L)
  conv_w (w=1..5): out_w[f, t] = sum_{h,dw} Ww[f,h,dw] * X[h, t+dw]
  pool_w[f] = relu(bias_w[f] + max_{t <= Leff-w} out_w[f, t])
  fc1 -> fc2 -> sigmoid

Kernel strategy (8 cores, uniform SPMD program, per-core data tables):
  - Sort the 128 samples by length desc; slot j holds ranks [8j, 8j+8), one
    per core.  Canonical slot width Wc[j] = max length in slot (baked into
    the program); each core's actual lengths live only in data (offset
    tables + masks), so one program serves all 8 cores.
  - Per sample, one indirect (gather) DMA builds SBUF tile
    F[p, k*Wc + j] = flat[(4p+k)*L + j] from 512 host-computed chunk
    offsets.  Matmul k-tile k of the contraction over h = rows h' = 4p+k,
    with conv weights pre-permuted on host to match.
  - conv w outputs accumulate in one PSUM bank per (slot, w): 4*w matmuls
    (dw-shift folded into the rhs column window) plus one K=1 matmul that
    adds -1e30 to invalid output columns (mask rows are host data), so
    pool = relu(bias + reduce_max(psum)).
  - Tiny fc1/fc2/sigmoid on-chip, output [1, 16] per core.
"""

import os
import sys

for _p in ("/opt/trn_rl_repo", "/root/.axon_site/_ro/trn_rl_repo"):
    if os.path.isdir(_p) and _p not in sys.path:
        sys.path.insert(0, _p)

import numpy as np
import ml_dtypes

T = 512
B = 128
H = 512
NF = 128
FS = 5
P = 128
NCORES = 8
NSLOT = B // NCORES  # 16
SAMP = T * H  # elements per sample block
MW = 520  # per-slot mask row width

USE_BF16 = True

LAST_EXEC_NS = None
LAST_RESULTS = None
_PROGRAM_CACHE = {}


def _pair_index(w, dw):
    # enumerate (w, dw) pairs: w=1..5, dw=0..w-1 -> 0..14
    return (w - 1) * w // 2 + dw


def build_program(Wc, use_bf16=True):
    import concourse.bass as bass
    import concourse.bacc as bacc
    import concourse.mybir as mybir
    from concourse.tile import TileContext

    f32 = mybir.dt.float32
    cdt = mybir.dt.bfloat16 if use_bf16 else f32
    i32 = mybir.dt.int32
    AX = mybir.AxisListType
    AF = mybir.ActivationFunctionType

    nc = bacc.Bacc()
    enc = nc.declare_dram_parameter("enc", [NSLOT * SAMP, 1], f32, isOutput=False)
    idx = nc.declare_dram_parameter("idx", [P, NSLOT * 4], i32, isOutput=False)
    # mask rows ++ [neg row] packed in one bf16 tensor
    msk = nc.declare_dram_parameter("msk", [1, NSLOT * MW + P], cdt, isOutput=False)
    wconv = nc.declare_dram_parameter("wconv", [P, 60 * P], cdt, isOutput=False)
    # cbias[:, :5] ++ fc1b (cols 5..) ++ fc2b (col 6 row 0) in one f32 tensor
    fcon = nc.declare_dram_parameter("fcon", [P, 7], f32, isOutput=False)
    # fc1w tiles ++ fc2w (col 500) in one bf16 tensor
    fcw = nc.declare_dram_parameter("fcw", [P, 5 * 100 + 1], cdt, isOutput=False)
    out = nc.declare_dram_parameter("out", [1, NSLOT], f32, isOutput=True)

    # process slots largest-first: each group's matmul span covers the next
    # group's gathers, and the canonical widths are sorted descending anyway
    order = sorted(range(NSLOT), key=lambda j: -Wc[j])
    groups = [order[i : i + 4] for i in range(0, NSLOT, 4)]

    with TileContext(nc) as tc:
        with (
            tc.tile_pool(name="const", bufs=1) as constp,
            tc.tile_pool(name="fpool", bufs=16) as fpool,
            tc.tile_pool(name="pspool", bufs=8, space="PSUM") as pspool,
        ):
            # load order matters: idx unblocks gathers, msk/neg unblock mask
            # matmuls, wconv unblocks the weight matmuls; fc consts at the end
            idx_sb = constp.tile([P, NSLOT * 4], i32, tag="idx")
            nc.sync.dma_start(out=idx_sb[:], in_=idx[:])
            mskneg_sb = constp.tile([1, NSLOT * MW + P], cdt, tag="msk")
            nc.sync.dma_start(out=mskneg_sb[:], in_=msk[:])
            msk_sb = mskneg_sb
            neg_sb = mskneg_sb[:, NSLOT * MW : NSLOT * MW + P]
            wsb = constp.tile([P, 60 * P], cdt, tag="wsb")
            nc.sync.dma_start(out=wsb[:], in_=wconv[:])
            fcon_sb = constp.tile([P, 7], f32, tag="fcon")
            nc.scalar.dma_start(out=fcon_sb[:], in_=fcon[:])
            cb_sb = fcon_sb[:, 0:FS]
            fc1b_sb = fcon_sb[:100, FS : FS + 1]
            fc2b_sb = fcon_sb[:1, FS + 1 : FS + 2]
            fcw_sb = constp.tile([P, 5 * 100 + 1], cdt, tag="fcw")
            nc.scalar.dma_start(out=fcw_sb[:], in_=fcw[:])
            fc1w_sb = fcw_sb[:, 0 : 5 * 100]
            fc2w_sb = fcw_sb[:100, 5 * 100 : 5 * 100 + 1]

            # pool results: pools[w-1] fp32 [128, NSLOT]; bf16 relu'd copies
            pools = []
            poolsr = []
            for w in range(1, FS + 1):
                pw = constp.tile([P, NSLOT], f32, tag=f"pool{w}", name=f"pool{w}")
                pr = constp.tile([P, NSLOT], cdt, tag=f"poolr{w}", name=f"poolr{w}")
                pools.append(pw)
                poolsr.append(pr)

            fts = {}
            for gi, grp in enumerate(groups):
                for j in grp:
                    ft = fpool.tile([P, 4 * Wc[j]], cdt, tag="F", name=f"ft{j}")
                    # HW indirect DMA consumes one index per dest partition row
                    for k in range(4):
                        nc.gpsimd.indirect_dma_start(
                            out=ft[:, k * Wc[j] : (k + 1) * Wc[j]],
                            out_offset=None,
                            in_=enc[:],
                            in_offset=bass.IndirectOffsetOnAxis(
                                ap=idx_sb[:, j * 4 + k : j * 4 + k + 1], axis=0
                            ),
                        )
                    fts[j] = ft
                for w in range(1, FS + 1):
                    pss = {}
                    for j in grp:
                        Nw = Wc[j] - w + 1
                        ps = pspool.tile([P, Nw], f32, tag="ps", name=f"ps{j}w{w}")
                        # mask matmul: psum[f, t] += -1e30 * M[j*MW + w + t]
                        nc.tensor.matmul(
                            ps[:],
                            neg_sb,
                            msk_sb[:, j * MW + w : j * MW + w + Nw],
                            start=True,
                            stop=False,
                        )
                        pss[j] = ps
                    for dw in range(w):
                        i = _pair_index(w, dw)
                        for k in range(4):
                            last = (dw == w - 1) and (k == 3)
                            for j in grp:
                                Nw = Wc[j] - w + 1
                                c0 = k * Wc[j] + dw
                                nc.tensor.matmul(
                                    pss[j][:],
                                    wsb[:, (i * 4 + k) * P : (i * 4 + k + 1) * P],
                                    fts[j][:, c0 : c0 + Nw],
                                    start=False,
                                    stop=last,
                                )
                    for j in grp:
                        nc.vector.reduce_max(
                            pools[w - 1][:, j : j + 1], pss[j][:], axis=AX.X
                        )

            # pool_w = relu(max + bias_w)
            for w in range(1, FS + 1):
                nc.scalar.activation(
                    poolsr[w - 1][:],
                    pools[w - 1][:],
                    AF.Relu,
                    bias=cb_sb[:, w - 1 : w],
                )

            psf1 = pspool.tile([100, NSLOT], f32, tag="ps", name="psf1")
            for k in range(5):
                nc.tensor.matmul(
                    psf1[:],
                    fc1w_sb[:, k * 100 : (k + 1) * 100],
                    poolsr[k][:],
                    start=(k == 0),
                    stop=(k == 4),
                )
            fc1_sb = constp.tile([100, NSLOT], cdt, tag="fc1o")
            nc.scalar.activation(fc1_sb[:], psf1[:], AF.Identity, bias=fc1b_sb)

            psf2 = pspool.tile([1, NSLOT], f32, tag="ps", name="psf2")
            nc.tensor.matmul(psf2[:], fc2w_sb, fc1_sb[:], start=True, stop=True)
            out_sb = constp.tile([1, NSLOT], f32, tag="outsb")
            nc.scalar.activation(out_sb[:], psf2[:], AF.Sigmoid, bias=fc2b_sb)
            nc.sync.dma_start(out=out[:], in_=out_sb[:])

    nc.compile()
    return nc


def prepare(encoder_output, lengths, conv_ws, conv_bs, fc1_w, fc1_b, fc2_w, fc2_b,
            use_bf16=None):
    """Host-side prep: sample assignment, per-core data tables, program build.

    Returns (nc, in_maps, assignment) where assignment[c][j] = global sample.
    """
    if use_bf16 is None:
        use_bf16 = USE_BF16
    enc = np.ascontiguousarray(np.asarray(encoder_output, dtype=np.float32))
    lens = np.asarray(lengths).astype(np.int64)
    assert enc.shape == (T, B, H)
    assert lens.shape == (B,)

    cdt = ml_dtypes.bfloat16 if use_bf16 else np.float32

    # effective lengths (L < FS samples get rebuilt blocks with L_eff = FS)
    eff = np.maximum(lens, FS)

    # sort desc by effective length; slot j <- ranks [8j, 8j+8)
    ranks = np.argsort(-eff, kind="stable")
    assignment = [[int(ranks[8 * j + c]) for j in range(NSLOT)] for c in range(NCORES)]
    Wc = tuple(int(eff[ranks[8 * j]]) for j in range(NSLOT))

    encT = enc.transpose(1, 0, 2)  # [B, T, H], sample-major views

    in_maps = []
    for c in range(NCORES):
        enc_c = np.empty((NSLOT, T, H), dtype=np.float32)
        idx_c = np.empty((P, NSLOT * 4), dtype=np.int32)
        msk_c = np.zeros((1, NSLOT * MW + P), dtype=np.float32)
        msk_c[0, NSLOT * MW :] = -1e30
        for j in range(NSLOT):
            b = assignment[c][j]
            L = int(lens[b])
            Le = int(eff[b])
            if L >= FS:
                enc_c[j] = encT[b]
            else:
                # rebuild: flat'[h*FS + jj] = flat[h*L + jj] for jj < L else 0
                blk = np.zeros((T, H), dtype=np.float32)
                flat = encT[b].reshape(-1)[: H * L]
                v = np.zeros((H, FS), dtype=np.float32)
                v[:, :L] = flat.reshape(H, L)
                blk.reshape(-1)[: H * FS] = v.reshape(-1)
                enc_c[j] = blk
            base = j * SAMP
            pk = np.arange(P)[:, None] * 4 + np.arange(4)[None, :]  # [128, 4]
            idx_c[:, j * 4 : (j + 1) * 4] = base + pk * Le
            u = np.arange(MW)
            msk_c[0, j * MW : (j + 1) * MW] = (u > Le).astype(np.float32)

        in_maps.append(
            {
                "enc": enc_c.reshape(NSLOT * SAMP, 1),
                "idx": idx_c,
                "msk": msk_c.astype(cdt),
            }
        )

    # weights, shared across cores
    wconv = np.empty((P, 60 * P), dtype=np.float32)
    hsel = np.arange(P)[:, None] * 4  # [128,1]
    for w in range(1, FS + 1):
        Ww = np.asarray(conv_ws[w - 1], dtype=np.float32)  # [NF, 1, H, w]
        for dw in range(w):
            i = _pair_index(w, dw)
            for k in range(4):
                # lhsT[p, f] = Ww[f, 0, 4p+k, dw]
                wconv[:, (i * 4 + k) * P : (i * 4 + k + 1) * P] = Ww[
                    :, 0, (hsel + k).ravel(), dw
                ].T
    fcon = np.zeros((P, 7), dtype=np.float32)
    fcon[:, 0:FS] = np.stack([np.asarray(b, dtype=np.float32) for b in conv_bs], axis=1)
    fcon[:100, FS] = np.asarray(fc1_b, dtype=np.float32)
    fcon[0, FS + 1] = np.float32(np.asarray(fc2_b, dtype=np.float32).reshape(-1)[0])
    fcw_host = np.zeros((P, 5 * 100 + 1), dtype=np.float32)
    fc1_w = np.asarray(fc1_w, dtype=np.float32)  # [100, 640]
    for k in range(5):
        fcw_host[:, k * 100 : (k + 1) * 100] = fc1_w[:, k * P : (k + 1) * P].T
    fcw_host[:100, 5 * 100] = np.asarray(fc2_w, dtype=np.float32).reshape(-1)
    shared = {
        "wconv": wconv.astype(cdt),
        "fcon": fcon,
        "fcw": fcw_host.astype(cdt),
    }
    for m in in_maps:
        m.update(shared)

    key = (Wc, use_bf16)
    if key not in _PROGRAM_CACHE:
        _PROGRAM_CACHE[key] = build_program(Wc, use_bf16)
    nc = _PROGRAM_CACHE[key]
    return nc, in_maps, assignment


def _ensure_ntff_hook():
    """Install the axon NTFF profile hook if the image's antenv lacks it."""
    import types

    try:
        from antenv.axon_hooks import get_axon_ntff_profile_hook  # noqa: F401
        return True
    except ImportError:
        pass
    try:
        import antenv
        from trn_agent_boot.trn_boot import _ntff_profile_via_ctypes

        hook = _ntff_profile_via_ctypes("/opt/axon/libaxon_pjrt.so")
        mod = types.ModuleType("antenv.axon_hooks")
        _state = {"hook": hook}
        mod.get_axon_ntff_profile_hook = lambda: _state["hook"]
        mod.set_axon_ntff_profile_hook = lambda h: _state.update(hook=h)
        sys.modules["antenv.axon_hooks"] = mod
        antenv.axon_hooks = mod
        return hook is not None
    except Exception as e:  # pragma: no cover
        print(f"ntff hook install failed: {e}", file=sys.stderr)
        return False


def kernel(encoder_output, lengths,
           conv_w1, conv_b1, conv_w2, conv_b2, conv_w3, conv_b3,
           conv_w4, conv_b4, conv_w5, conv_b5,
           fc1_w, fc1_b, fc2_w, fc2_b):
    global LAST_EXEC_NS, LAST_RESULTS
    from concourse.bass_utils import run_bass_kernel_spmd

    conv_ws = [conv_w1, conv_w2, conv_w3, conv_w4, conv_w5]
    conv_bs = [conv_b1, conv_b2, conv_b3, conv_b4, conv_b5]
    nc, in_maps, assignment = prepare(
        encoder_output, lengths, conv_ws, conv_bs, fc1_w, fc1_b, fc2_w, fc2_b
    )

    trace = bool(int(os.environ.get("KERNEL_TRACE", "0")))
    if trace:
        trace = _ensure_ntff_hook()
    res = run_bass_kernel_spmd(nc, in_maps, list(range(NCORES)), trace=trace)
    LAST_RESULTS = res
    LAST_EXEC_NS = getattr(res, "exec_time_ns", None)

    out_full = np.empty((B, 1, 1), dtype=np.float32)
    for c in range(NCORES):
        oc = np.asarray(res.results[c]["out"]).reshape(NSLOT)
        for j in range(NSLOT):
            out_full[assignment[c][j], 0, 0] = oc[j]
    return out_full


# revision 20
# speedup vs baseline: 1.2425x; 1.1005x over previous
"""Trainium2 Bass kernel for nn_ConvDiscriminator (ragged CNN discriminator).

Math (per sample b with length L):
  flat = encoder_output[0:L, b, :].ravel()           # contiguous [L*512]
  X[h, l] = flat[h*L + l]  (raw reshape to [512, L], zero-pad cols >= L)
  conv_w (w=1..5): out_w[f, t] = sum_{h,dw} Ww[f,h,dw] * X[h, t+dw]
  pool_w[f] = relu(bias_w[f] + max_{t <= Leff-w} out_w[f, t])
  fc1 -> fc2 -> sigmoid

Kernel strategy (8 cores, uniform SPMD program, per-core data tables):
  - Sort the 128 samples by length desc; slot j holds ranks [8j, 8j+8), one
    per core.  Canonical slot width Wc[j] = max length in slot (baked into
    the program); each core's actual lengths live only in data (offset
    tables + masks), so one program serves all 8 cores.
  - Per sample, one indirect (gather) DMA builds SBUF tile
    F[p, k*Wc + j] = flat[(4p+k)*L + j] from 512 host-computed chunk
    offsets.  Matmul k-tile k of the contraction over h = rows h' = 4p+k,
    with conv weights pre-permuted on host to match.
  - conv w outputs accumulate in one PSUM bank per (slot, w): 4*w matmuls
    (dw-shift folded into the rhs column window) plus one K=1 matmul that
    adds -1e30 to invalid output columns (mask rows are host data), so
    pool = relu(bias + reduce_max(psum)).
  - Tiny fc1/fc2/sigmoid on-chip, output [1, 16] per core.
"""

import os
import sys

for _p in ("/opt/trn_rl_repo", "/root/.axon_site/_ro/trn_rl_repo"):
    if os.path.isdir(_p) and _p not in sys.path:
        sys.path.insert(0, _p)

import numpy as np
import ml_dtypes

T = 512
B = 128
H = 512
NF = 128
FS = 5
P = 128
NCORES = 8
NSLOT = B // NCORES  # 16
SAMP = T * H  # elements per sample block
MW = 520  # per-slot mask row width

USE_BF16 = True

LAST_EXEC_NS = None
LAST_RESULTS = None
_PROGRAM_CACHE = {}


def _pair_index(w, dw):
    # enumerate (w, dw) pairs: w=1..5, dw=0..w-1 -> 0..14
    return (w - 1) * w // 2 + dw


def build_program(Wc, use_bf16=True):
    import concourse.bass as bass
    import concourse.bacc as bacc
    import concourse.mybir as mybir
    from concourse.tile import TileContext

    f32 = mybir.dt.float32
    cdt = mybir.dt.bfloat16 if use_bf16 else f32
    i32 = mybir.dt.int32
    AX = mybir.AxisListType
    AF = mybir.ActivationFunctionType

    nc = bacc.Bacc()
    enc = nc.declare_dram_parameter("enc", [NSLOT * SAMP, 1], f32, isOutput=False)
    idx = nc.declare_dram_parameter("idx", [P, NSLOT * 4], i32, isOutput=False)
    # mask rows on partitions {0,32,64,96} (4 slots of a group side by side,
    # one column-block per group) ++ neg rows at columns [4*MW, 4*MW+P)
    msk = nc.declare_dram_parameter("msk", [P, 4 * MW + P], cdt, isOutput=False)
    wconv = nc.declare_dram_parameter("wconv", [P, 60 * P], cdt, isOutput=False)
    # cbias[:, :5] ++ fc1b (cols 5..) ++ fc2b (col 6 row 0) in one f32 tensor
    fcon = nc.declare_dram_parameter("fcon", [P, 7], f32, isOutput=False)
    # fc1w tiles ++ fc2w (col 500) in one bf16 tensor
    fcw = nc.declare_dram_parameter("fcw", [P, 5 * 100 + 1], cdt, isOutput=False)
    out = nc.declare_dram_parameter("out", [1, NSLOT], f32, isOutput=True)

    # process slots largest-first: each group's matmul span covers the next
    # group's gathers, and the canonical widths are sorted descending anyway
    order = sorted(range(NSLOT), key=lambda j: -Wc[j])
    groups = [order[i : i + 4] for i in range(0, NSLOT, 4)]

    with TileContext(nc) as tc:
        with (
            tc.tile_pool(name="const", bufs=1) as constp,
            tc.tile_pool(name="fpool", bufs=16) as fpool,
            tc.tile_pool(name="pspool", bufs=8, space="PSUM") as pspool,
        ):
            # load order matters: idx unblocks gathers, msk/neg unblock mask
            # matmuls, wconv unblocks the weight matmuls; fc consts at the end
            idx_sb = constp.tile([P, NSLOT * 4], i32, tag="idx")
            nc.sync.dma_start(out=idx_sb[:], in_=idx[:])
            msk_sb = constp.tile([P, 4 * MW + P], cdt, tag="msk")
            nc.sync.dma_start(out=msk_sb[:], in_=msk[:])
            wsb = constp.tile([P, 60 * P], cdt, tag="wsb")
            nc.sync.dma_start(out=wsb[:], in_=wconv[:])
            fcon_sb = constp.tile([P, 7], f32, tag="fcon")
            nc.scalar.dma_start(out=fcon_sb[:], in_=fcon[:])
            cb_sb = fcon_sb[:, 0:FS]
            fc1b_sb = fcon_sb[:100, FS : FS + 1]
            fc2b_sb = fcon_sb[:1, FS + 1 : FS + 2]
            fcw_sb = constp.tile([P, 5 * 100 + 1], cdt, tag="fcw")
            nc.scalar.dma_start(out=fcw_sb[:], in_=fcw[:])
            fc1w_sb = fcw_sb[:, 0 : 5 * 100]
            fc2w_sb = fcw_sb[:100, 5 * 100 : 5 * 100 + 1]

            # pool results: pools[w-1] fp32 [128, NSLOT]; bf16 relu'd copies
            pools = []
            poolsr = []
            for w in range(1, FS + 1):
                pw = constp.tile([P, NSLOT], f32, tag=f"pool{w}", name=f"pool{w}")
                pr = constp.tile([P, NSLOT], cdt, tag=f"poolr{w}", name=f"poolr{w}")
                pools.append(pw)
                poolsr.append(pr)

            fts = {}
            for gi, grp in enumerate(groups):
                for j in grp:
                    ft = fpool.tile([P, 4 * Wc[j]], cdt, tag="F", name=f"ft{j}")
                    # HW indirect DMA consumes one index per dest partition row
                    for k in range(4):
                        nc.gpsimd.indirect_dma_start(
                            out=ft[:, k * Wc[j] : (k + 1) * Wc[j]],
                            out_offset=None,
                            in_=enc[:],
                            in_offset=bass.IndirectOffsetOnAxis(
                                ap=idx_sb[:, j * 4 + k : j * 4 + k + 1], axis=0
                            ),
                        )
                    fts[j] = ft
                def mask_mm(ps, ii, w, Nw):
                    # psum[f, t] += -1e30 * M[t + w]; mask/neg rows for group
                    # member ii live on partition 32*ii; pack via tile_position
                    q = 32 * ii
                    nc.tensor.matmul(
                        ps[:],
                        msk_sb[q : q + 1, 4 * MW : 4 * MW + P],
                        msk_sb[q : q + 1, gi * MW + w : gi * MW + w + Nw],
                        start=True,
                        stop=False,
                        tile_position=(q, 0),
                    )

                def weight_mm(ps, j, w, dw, k):
                    Nw = Wc[j] - w + 1
                    i = _pair_index(w, dw)
                    c0 = k * Wc[j] + dw
                    nc.tensor.matmul(
                        ps[:],
                        wsb[:, (i * 4 + k) * P : (i * 4 + k + 1) * P],
                        fts[j][:, c0 : c0 + Nw],
                        start=False,
                        stop=(dw == w - 1) and (k == 3),
                    )

                if gi == 0:
                    # slot-major: start crunching slot j right after its gather
                    for ii, j in enumerate(grp):
                        for w in range(1, FS + 1):
                            Nw = Wc[j] - w + 1
                            ps = pspool.tile([P, Nw], f32, tag="ps", name=f"ps{j}w{w}")
                            mask_mm(ps, ii, w, Nw)
                            for dw in range(w):
                                for k in range(4):
                                    weight_mm(ps, j, w, dw, k)
                            nc.vector.reduce_max(
                                pools[w - 1][:, j : j + 1], ps[:], axis=AX.X
                            )
                else:
                    for w in range(1, FS + 1):
                        pss = {}
                        for ii, j in enumerate(grp):
                            Nw = Wc[j] - w + 1
                            ps = pspool.tile([P, Nw], f32, tag="ps", name=f"ps{j}w{w}")
                            mask_mm(ps, ii, w, Nw)
                            pss[j] = ps
                        for dw in range(w):
                            for k in range(4):
                                for j in grp:
                                    weight_mm(pss[j], j, w, dw, k)
                        for j in grp:
                            nc.vector.reduce_max(
                                pools[w - 1][:, j : j + 1], pss[j][:], axis=AX.X
                            )

            # pool_w = relu(max + bias_w)
            for w in range(1, FS + 1):
                nc.scalar.activation(
                    poolsr[w - 1][:],
                    pools[w - 1][:],
                    AF.Relu,
                    bias=cb_sb[:, w - 1 : w],
                )

            psf1 = pspool.tile([100, NSLOT], f32, tag="ps", name="psf1")
            for k in range(5):
                nc.tensor.matmul(
                    psf1[:],
                    fc1w_sb[:, k * 100 : (k + 1) * 100],
                    poolsr[k][:],
                    start=(k == 0),
                    stop=(k == 4),
                )
            fc1_sb = constp.tile([100, NSLOT], cdt, tag="fc1o")
            nc.scalar.activation(fc1_sb[:], psf1[:], AF.Identity, bias=fc1b_sb)

            psf2 = pspool.tile([1, NSLOT], f32, tag="ps", name="psf2")
            nc.tensor.matmul(psf2[:], fc2w_sb, fc1_sb[:], start=True, stop=True)
            out_sb = constp.tile([1, NSLOT], f32, tag="outsb")
            nc.scalar.activation(out_sb[:], psf2[:], AF.Sigmoid, bias=fc2b_sb)
            nc.sync.dma_start(out=out[:], in_=out_sb[:])

    nc.compile()
    return nc


def prepare(encoder_output, lengths, conv_ws, conv_bs, fc1_w, fc1_b, fc2_w, fc2_b,
            use_bf16=None):
    """Host-side prep: sample assignment, per-core data tables, program build.

    Returns (nc, in_maps, assignment) where assignment[c][j] = global sample.
    """
    if use_bf16 is None:
        use_bf16 = USE_BF16
    enc = np.ascontiguousarray(np.asarray(encoder_output, dtype=np.float32))
    lens = np.asarray(lengths).astype(np.int64)
    assert enc.shape == (T, B, H)
    assert lens.shape == (B,)

    cdt = ml_dtypes.bfloat16 if use_bf16 else np.float32

    # effective lengths (L < FS samples get rebuilt blocks with L_eff = FS)
    eff = np.maximum(lens, FS)

    # sort desc by effective length; slot j <- ranks [8j, 8j+8)
    ranks = np.argsort(-eff, kind="stable")
    assignment = [[int(ranks[8 * j + c]) for j in range(NSLOT)] for c in range(NCORES)]
    Wc = tuple(int(eff[ranks[8 * j]]) for j in range(NSLOT))

    encT = enc.transpose(1, 0, 2)  # [B, T, H], sample-major views

    # (group, member) position of each slot — must match build_program
    order = sorted(range(NSLOT), key=lambda j: -Wc[j])
    slot_pos = {}
    for g in range(4):
        for i in range(4):
            slot_pos[order[4 * g + i]] = (g, i)

    in_maps = []
    for c in range(NCORES):
        enc_c = np.empty((NSLOT, T, H), dtype=np.float32)
        idx_c = np.empty((P, NSLOT * 4), dtype=np.int32)
        msk_c = np.zeros((P, 4 * MW + P), dtype=np.float32)
        msk_c[::32, 4 * MW :] = -1e30
        for j in range(NSLOT):
            b = assignment[c][j]
            L = int(lens[b])
            Le = int(eff[b])
            if L >= FS:
                enc_c[j] = encT[b]
            else:
                # rebuild: flat'[h*FS + jj] = flat[h*L + jj] for jj < L else 0
                blk = np.zeros((T, H), dtype=np.float32)
                flat = encT[b].reshape(-1)[: H * L]
                v = np.zeros((H, FS), dtype=np.float32)
                v[:, :L] = flat.reshape(H, L)
                blk.reshape(-1)[: H * FS] = v.reshape(-1)
                enc_c[j] = blk
            base = j * SAMP
            pk = np.arange(P)[:, None] * 4 + np.arange(4)[None, :]  # [128, 4]
            idx_c[:, j * 4 : (j + 1) * 4] = base + pk * Le
            g, i = slot_pos[j]
            u = np.arange(MW)
            msk_c[32 * i, g * MW : (g + 1) * MW] = (u > Le).astype(np.float32)

        in_maps.append(
            {
                "enc": enc_c.reshape(NSLOT * SAMP, 1),
                "idx": idx_c,
                "msk": msk_c.astype(cdt),
            }
        )

    # weights, shared across cores
    wconv = np.empty((P, 60 * P), dtype=np.float32)
    hsel = np.arange(P)[:, None] * 4  # [128,1]
    for w in range(1, FS + 1):
        Ww = np.asarray(conv_ws[w - 1], dtype=np.float32)  # [NF, 1, H, w]
        for dw in range(w):
            i = _pair_index(w, dw)
            for k in range(4):
                # lhsT[p, f] = Ww[f, 0, 4p+k, dw]
                wconv[:, (i * 4 + k) * P : (i * 4 + k + 1) * P] = Ww[
                    :, 0, (hsel + k).ravel(), dw
                ].T
    fcon = np.zeros((P, 7), dtype=np.float32)
    fcon[:, 0:FS] = np.stack([np.asarray(b, dtype=np.float32) for b in conv_bs], axis=1)
    fcon[:100, FS] = np.asarray(fc1_b, dtype=np.float32)
    fcon[0, FS + 1] = np.float32(np.asarray(fc2_b, dtype=np.float32).reshape(-1)[0])
    fcw_host = np.zeros((P, 5 * 100 + 1), dtype=np.float32)
    fc1_w = np.asarray(fc1_w, dtype=np.float32)  # [100, 640]
    for k in range(5):
        fcw_host[:, k * 100 : (k + 1) * 100] = fc1_w[:, k * P : (k + 1) * P].T
    fcw_host[:100, 5 * 100] = np.asarray(fc2_w, dtype=np.float32).reshape(-1)
    shared = {
        "wconv": wconv.astype(cdt),
        "fcon": fcon,
        "fcw": fcw_host.astype(cdt),
    }
    for m in in_maps:
        m.update(shared)

    key = (Wc, use_bf16)
    if key not in _PROGRAM_CACHE:
        _PROGRAM_CACHE[key] = build_program(Wc, use_bf16)
    nc = _PROGRAM_CACHE[key]
    return nc, in_maps, assignment


def _ensure_ntff_hook():
    """Install the axon NTFF profile hook if the image's antenv lacks it."""
    import types

    try:
        from antenv.axon_hooks import get_axon_ntff_profile_hook  # noqa: F401
        return True
    except ImportError:
        pass
    try:
        import antenv
        from trn_agent_boot.trn_boot import _ntff_profile_via_ctypes

        hook = _ntff_profile_via_ctypes("/opt/axon/libaxon_pjrt.so")
        mod = types.ModuleType("antenv.axon_hooks")
        _state = {"hook": hook}
        mod.get_axon_ntff_profile_hook = lambda: _state["hook"]
        mod.set_axon_ntff_profile_hook = lambda h: _state.update(hook=h)
        sys.modules["antenv.axon_hooks"] = mod
        antenv.axon_hooks = mod
        return hook is not None
    except Exception as e:  # pragma: no cover
        print(f"ntff hook install failed: {e}", file=sys.stderr)
        return False


def kernel(encoder_output, lengths,
           conv_w1, conv_b1, conv_w2, conv_b2, conv_w3, conv_b3,
           conv_w4, conv_b4, conv_w5, conv_b5,
           fc1_w, fc1_b, fc2_w, fc2_b):
    global LAST_EXEC_NS, LAST_RESULTS
    from concourse.bass_utils import run_bass_kernel_spmd

    conv_ws = [conv_w1, conv_w2, conv_w3, conv_w4, conv_w5]
    conv_bs = [conv_b1, conv_b2, conv_b3, conv_b4, conv_b5]
    nc, in_maps, assignment = prepare(
        encoder_output, lengths, conv_ws, conv_bs, fc1_w, fc1_b, fc2_w, fc2_b
    )

    trace = bool(int(os.environ.get("KERNEL_TRACE", "0")))
    if trace:
        trace = _ensure_ntff_hook()
    res = run_bass_kernel_spmd(nc, in_maps, list(range(NCORES)), trace=trace)
    LAST_RESULTS = res
    LAST_EXEC_NS = getattr(res, "exec_time_ns", None)

    out_full = np.empty((B, 1, 1), dtype=np.float32)
    for c in range(NCORES):
        oc = np.asarray(res.results[c]["out"]).reshape(NSLOT)
        for j in range(NSLOT):
            out_full[assignment[c][j], 0, 0] = oc[j]
    return out_full


# revision 22
# speedup vs baseline: 1.4275x; 1.1489x over previous
"""Trainium2 Bass kernel for nn_ConvDiscriminator (ragged CNN discriminator).

Math (per sample b with length L):
  flat = encoder_output[0:L, b, :].ravel()           # contiguous [L*512]
  X[h, l] = flat[h*L + l]  (raw reshape to [512, L], zero-pad cols >= L)
  conv_w (w=1..5): out_w[f, t] = sum_{h,dw} Ww[f,h,dw] * X[h, t+dw]
  pool_w[f] = relu(bias_w[f] + max_{t <= Leff-w} out_w[f, t])
  fc1 -> fc2 -> sigmoid

Kernel strategy (8 cores, uniform SPMD program, per-core data tables):
  - Sort the 128 samples by length desc; slot j holds ranks [8j, 8j+8), one
    per core.  Canonical slot width Wc[j] = max length in slot (baked into
    the program); each core's actual lengths live only in data (offset
    tables + masks), so one program serves all 8 cores.
  - Per sample, one indirect (gather) DMA builds SBUF tile
    F[p, k*Wc + j] = flat[(4p+k)*L + j] from 512 host-computed chunk
    offsets.  Matmul k-tile k of the contraction over h = rows h' = 4p+k,
    with conv weights pre-permuted on host to match.
  - conv w outputs accumulate in one PSUM bank per (slot, w): 4*w matmuls
    (dw-shift folded into the rhs column window) plus one K=1 matmul that
    adds -1e30 to invalid output columns (mask rows are host data), so
    pool = relu(bias + reduce_max(psum)).
  - Tiny fc1/fc2/sigmoid on-chip, output [1, 16] per core.
"""

import os
import sys

for _p in ("/opt/trn_rl_repo", "/root/.axon_site/_ro/trn_rl_repo"):
    if os.path.isdir(_p) and _p not in sys.path:
        sys.path.insert(0, _p)

import numpy as np
import ml_dtypes

T = 512
B = 128
H = 512
NF = 128
FS = 5
P = 128
NCORES = 8
NSLOT = B // NCORES  # 16
SAMP = T * H  # elements per sample block
MW = 520  # per-slot mask row width

USE_BF16 = True
USE_FP8 = True  # fp8e4m3 DoubleRow conv path (masks/fc stay bf16)

LAST_EXEC_NS = None
LAST_RESULTS = None
_PROGRAM_CACHE = {}


def _pair_index(w, dw):
    # enumerate (w, dw) pairs: w=1..5, dw=0..w-1 -> 0..14
    return (w - 1) * w // 2 + dw


def build_program(Wc, use_bf16=True, use_fp8=False):
    import concourse.bass as bass
    import concourse.bacc as bacc
    import concourse.mybir as mybir
    from concourse.tile import TileContext

    f32 = mybir.dt.float32
    cdt = mybir.dt.bfloat16 if use_bf16 else f32
    wdt = mybir.dt.float8e4 if use_fp8 else cdt  # conv weights + F tiles
    i32 = mybir.dt.int32
    AX = mybir.AxisListType
    AF = mybir.ActivationFunctionType

    nc = bacc.Bacc()
    enc = nc.declare_dram_parameter("enc", [NSLOT * SAMP, 1], f32, isOutput=False)
    idx = nc.declare_dram_parameter("idx", [P, NSLOT * 4], i32, isOutput=False)
    # mask rows on partitions {0,32,64,96} (4 slots of a group side by side,
    # one column-block per group) ++ neg rows at columns [4*MW, 4*MW+P)
    msk = nc.declare_dram_parameter("msk", [P, 4 * MW + P], cdt, isOutput=False)
    wconv = nc.declare_dram_parameter("wconv", [P, 60 * P], wdt, isOutput=False)
    # cbias[:, :5] ++ fc1b (cols 5..) ++ fc2b (col 6 row 0) in one f32 tensor
    fcon = nc.declare_dram_parameter("fcon", [P, 7], f32, isOutput=False)
    # fc1w tiles ++ fc2w (col 500) in one bf16 tensor
    fcw = nc.declare_dram_parameter("fcw", [P, 5 * 100 + 1], cdt, isOutput=False)
    out = nc.declare_dram_parameter("out", [1, NSLOT], f32, isOutput=True)

    # process slots largest-first: each group's matmul span covers the next
    # group's gathers, and the canonical widths are sorted descending anyway
    order = sorted(range(NSLOT), key=lambda j: -Wc[j])
    groups = [order[i : i + 4] for i in range(0, NSLOT, 4)]

    with TileContext(nc) as tc:
        with (
            tc.tile_pool(name="const", bufs=1) as constp,
            tc.tile_pool(name="fpool", bufs=16) as fpool,
            tc.tile_pool(name="pspool", bufs=8, space="PSUM") as pspool,
        ):
            # load order matters: idx unblocks gathers, msk/neg unblock mask
            # matmuls, wconv unblocks the weight matmuls; fc consts at the end
            idx_sb = constp.tile([P, NSLOT * 4], i32, tag="idx")
            nc.sync.dma_start(out=idx_sb[:], in_=idx[:])
            msk_sb = constp.tile([P, 4 * MW + P], cdt, tag="msk")
            nc.sync.dma_start(out=msk_sb[:], in_=msk[:])
            wsb = constp.tile([P, 60 * P], wdt, tag="wsb")
            nc.sync.dma_start(out=wsb[:], in_=wconv[:])
            fcon_sb = constp.tile([P, 7], f32, tag="fcon")
            nc.scalar.dma_start(out=fcon_sb[:], in_=fcon[:])
            cb_sb = fcon_sb[:, 0:FS]
            fc1b_sb = fcon_sb[:100, FS : FS + 1]
            fc2b_sb = fcon_sb[:1, FS + 1 : FS + 2]
            fcw_sb = constp.tile([P, 5 * 100 + 1], cdt, tag="fcw")
            nc.scalar.dma_start(out=fcw_sb[:], in_=fcw[:])
            fc1w_sb = fcw_sb[:, 0 : 5 * 100]
            fc2w_sb = fcw_sb[:100, 5 * 100 : 5 * 100 + 1]

            # pool results: pools[w-1] fp32 [128, NSLOT]; bf16 relu'd copies
            pools = []
            poolsr = []
            for w in range(1, FS + 1):
                pw = constp.tile([P, NSLOT], f32, tag=f"pool{w}", name=f"pool{w}")
                pr = constp.tile([P, NSLOT], cdt, tag=f"poolr{w}", name=f"poolr{w}")
                pools.append(pw)
                poolsr.append(pr)

            fts = {}
            for gi, grp in enumerate(groups):
                for j in grp:
                    ft = fpool.tile([P, 4 * Wc[j]], wdt, tag="F", name=f"ft{j}")
                    # HW indirect DMA consumes one index per dest partition row
                    for k in range(4):
                        nc.gpsimd.indirect_dma_start(
                            out=ft[:, k * Wc[j] : (k + 1) * Wc[j]],
                            out_offset=None,
                            in_=enc[:],
                            in_offset=bass.IndirectOffsetOnAxis(
                                ap=idx_sb[:, j * 4 + k : j * 4 + k + 1], axis=0
                            ),
                        )
                    fts[j] = ft
                def mask_mm(ps, ii, w, Nw):
                    # psum[f, t] += -1e30 * M[t + w]; mask/neg rows for group
                    # member ii live on partition 32*ii; pack via tile_position
                    q = 32 * ii
                    nc.tensor.matmul(
                        ps[:],
                        msk_sb[q : q + 1, 4 * MW : 4 * MW + P],
                        msk_sb[q : q + 1, gi * MW + w : gi * MW + w + Nw],
                        start=True,
                        stop=False,
                        tile_position=(q, 0),
                    )

                def weight_mm(ps, j, w, dw, k):
                    Nw = Wc[j] - w + 1
                    i = _pair_index(w, dw)
                    c0 = k * Wc[j] + dw
                    nc.tensor.matmul(
                        ps[:],
                        wsb[:, (i * 4 + k) * P : (i * 4 + k + 1) * P],
                        fts[j][:, c0 : c0 + Nw],
                        start=False,
                        stop=(dw == w - 1) and (k == 3),
                    )

                def weight_mm8(ps, j, w, dw, k0):
                    # fp8 DoubleRow: one matmul contracts k-tiles (k0, k0+1)
                    Nw = Wc[j] - w + 1
                    i = _pair_index(w, dw)
                    nc.tensor.matmul(
                        ps[:],
                        wsb[:].rearrange("p (k m) -> p k m", k=60)[
                            :, i * 4 + k0 : i * 4 + k0 + 2, :
                        ],
                        fts[j][:].rearrange("p (k w) -> p k w", k=4)[
                            :, k0 : k0 + 2, dw : dw + Nw
                        ],
                        start=False,
                        stop=(dw == w - 1) and (k0 == 2),
                        perf_mode=mybir.MatmulPerfMode.DoubleRow,
                    )

                ksteps = (0, 2) if use_fp8 else (0, 1, 2, 3)
                wmm = weight_mm8 if use_fp8 else weight_mm

                if gi == 0:
                    # slot-major: start crunching slot j right after its gather
                    for ii, j in enumerate(grp):
                        for w in range(1, FS + 1):
                            Nw = Wc[j] - w + 1
                            ps = pspool.tile([P, Nw], f32, tag="ps", name=f"ps{j}w{w}")
                            mask_mm(ps, ii, w, Nw)
                            for dw in range(w):
                                for k in ksteps:
                                    wmm(ps, j, w, dw, k)
                            nc.vector.reduce_max(
                                pools[w - 1][:, j : j + 1], ps[:], axis=AX.X
                            )
                else:
                    for w in range(1, FS + 1):
                        pss = {}
                        for ii, j in enumerate(grp):
                            Nw = Wc[j] - w + 1
                            ps = pspool.tile([P, Nw], f32, tag="ps", name=f"ps{j}w{w}")
                            mask_mm(ps, ii, w, Nw)
                            pss[j] = ps
                        for dw in range(w):
                            for k in ksteps:
                                for j in grp:
                                    wmm(pss[j], j, w, dw, k)
                        for j in grp:
                            nc.vector.reduce_max(
                                pools[w - 1][:, j : j + 1], pss[j][:], axis=AX.X
                            )

            # pool_w = relu(max + bias_w)
            for w in range(1, FS + 1):
                nc.scalar.activation(
                    poolsr[w - 1][:],
                    pools[w - 1][:],
                    AF.Relu,
                    bias=cb_sb[:, w - 1 : w],
                )

            psf1 = pspool.tile([100, NSLOT], f32, tag="ps", name="psf1")
            for k in range(5):
                nc.tensor.matmul(
                    psf1[:],
                    fc1w_sb[:, k * 100 : (k + 1) * 100],
                    poolsr[k][:],
                    start=(k == 0),
                    stop=(k == 4),
                )
            fc1_sb = constp.tile([100, NSLOT], cdt, tag="fc1o")
            nc.scalar.activation(fc1_sb[:], psf1[:], AF.Identity, bias=fc1b_sb)

            psf2 = pspool.tile([1, NSLOT], f32, tag="ps", name="psf2")
            nc.tensor.matmul(psf2[:], fc2w_sb, fc1_sb[:], start=True, stop=True)
            out_sb = constp.tile([1, NSLOT], f32, tag="outsb")
            nc.scalar.activation(out_sb[:], psf2[:], AF.Sigmoid, bias=fc2b_sb)
            nc.sync.dma_start(out=out[:], in_=out_sb[:])

    nc.compile()
    return nc


def prepare(encoder_output, lengths, conv_ws, conv_bs, fc1_w, fc1_b, fc2_w, fc2_b,
            use_bf16=None):
    """Host-side prep: sample assignment, per-core data tables, program build.

    Returns (nc, in_maps, assignment) where assignment[c][j] = global sample.
    """
    if use_bf16 is None:
        use_bf16 = USE_BF16
    use_fp8 = USE_FP8
    enc = np.ascontiguousarray(np.asarray(encoder_output, dtype=np.float32))
    lens = np.asarray(lengths).astype(np.int64)
    assert enc.shape == (T, B, H)
    assert lens.shape == (B,)

    cdt = ml_dtypes.bfloat16 if use_bf16 else np.float32

    # effective lengths (L < FS samples get rebuilt blocks with L_eff = FS)
    eff = np.maximum(lens, FS)

    # sort desc by effective length; slot j <- ranks [8j, 8j+8)
    ranks = np.argsort(-eff, kind="stable")
    assignment = [[int(ranks[8 * j + c]) for j in range(NSLOT)] for c in range(NCORES)]
    if use_fp8:
        # DoubleRow rhs pair-step must be 16-byte aligned -> widths % 16 == 0
        Wc = tuple(min(512, -(-int(eff[ranks[8 * j]]) // 16) * 16) for j in range(NSLOT))
    else:
        Wc = tuple(int(eff[ranks[8 * j]]) for j in range(NSLOT))

    encT = enc.transpose(1, 0, 2)  # [B, T, H], sample-major views

    # (group, member) position of each slot — must match build_program
    order = sorted(range(NSLOT), key=lambda j: -Wc[j])
    slot_pos = {}
    for g in range(4):
        for i in range(4):
            slot_pos[order[4 * g + i]] = (g, i)

    in_maps = []
    for c in range(NCORES):
        enc_c = np.empty((NSLOT, T, H), dtype=np.float32)
        idx_c = np.empty((P, NSLOT * 4), dtype=np.int32)
        msk_c = np.zeros((P, 4 * MW + P), dtype=np.float32)
        msk_c[::32, 4 * MW :] = -1e30
        for j in range(NSLOT):
            b = assignment[c][j]
            L = int(lens[b])
            Le = int(eff[b])
            if L >= FS:
                enc_c[j] = encT[b]
            else:
                # rebuild: flat'[h*FS + jj] = flat[h*L + jj] for jj < L else 0
                blk = np.zeros((T, H), dtype=np.float32)
                flat = encT[b].reshape(-1)[: H * L]
                v = np.zeros((H, FS), dtype=np.float32)
                v[:, :L] = flat.reshape(H, L)
                blk.reshape(-1)[: H * FS] = v.reshape(-1)
                enc_c[j] = blk
            base = j * SAMP
            pk = np.arange(P)[:, None] * 4 + np.arange(4)[None, :]  # [128, 4]
            idx_c[:, j * 4 : (j + 1) * 4] = base + pk * Le
            g, i = slot_pos[j]
            u = np.arange(MW)
            msk_c[32 * i, g * MW : (g + 1) * MW] = (u > Le).astype(np.float32)

        in_maps.append(
            {
                "enc": enc_c.reshape(NSLOT * SAMP, 1),
                "idx": idx_c,
                "msk": msk_c.astype(cdt),
            }
        )

    # weights, shared across cores
    wconv = np.empty((P, 60 * P), dtype=np.float32)
    hsel = np.arange(P)[:, None] * 4  # [128,1]
    for w in range(1, FS + 1):
        Ww = np.asarray(conv_ws[w - 1], dtype=np.float32)  # [NF, 1, H, w]
        for dw in range(w):
            i = _pair_index(w, dw)
            for k in range(4):
                # lhsT[p, f] = Ww[f, 0, 4p+k, dw]
                wconv[:, (i * 4 + k) * P : (i * 4 + k + 1) * P] = Ww[
                    :, 0, (hsel + k).ravel(), dw
                ].T
    fcon = np.zeros((P, 7), dtype=np.float32)
    fcon[:, 0:FS] = np.stack([np.asarray(b, dtype=np.float32) for b in conv_bs], axis=1)
    fcon[:100, FS] = np.asarray(fc1_b, dtype=np.float32)
    fcon[0, FS + 1] = np.float32(np.asarray(fc2_b, dtype=np.float32).reshape(-1)[0])
    fcw_host = np.zeros((P, 5 * 100 + 1), dtype=np.float32)
    fc1_w = np.asarray(fc1_w, dtype=np.float32)  # [100, 640]
    for k in range(5):
        fcw_host[:, k * 100 : (k + 1) * 100] = fc1_w[:, k * P : (k + 1) * P].T
    fcw_host[:100, 5 * 100] = np.asarray(fc2_w, dtype=np.float32).reshape(-1)
    shared = {
        "wconv": wconv.astype(ml_dtypes.float8_e4m3 if use_fp8 else cdt),
        "fcon": fcon,
        "fcw": fcw_host.astype(cdt),
    }
    for m in in_maps:
        m.update(shared)

    key = (Wc, use_bf16, use_fp8)
    if key not in _PROGRAM_CACHE:
        _PROGRAM_CACHE[key] = build_program(Wc, use_bf16, use_fp8)
    nc = _PROGRAM_CACHE[key]
    return nc, in_maps, assignment


def _ensure_ntff_hook():
    """Install the axon NTFF profile hook if the image's antenv lacks it."""
    import types

    try:
        from antenv.axon_hooks import get_axon_ntff_profile_hook  # noqa: F401
        return True
    except ImportError:
        pass
    try:
        import antenv
        from trn_agent_boot.trn_boot import _ntff_profile_via_ctypes

        hook = _ntff_profile_via_ctypes("/opt/axon/libaxon_pjrt.so")
        mod = types.ModuleType("antenv.axon_hooks")
        _state = {"hook": hook}
        mod.get_axon_ntff_profile_hook = lambda: _state["hook"]
        mod.set_axon_ntff_profile_hook = lambda h: _state.update(hook=h)
        sys.modules["antenv.axon_hooks"] = mod
        antenv.axon_hooks = mod
        return hook is not None
    except Exception as e:  # pragma: no cover
        print(f"ntff hook install failed: {e}", file=sys.stderr)
        return False


def kernel(encoder_output, lengths,
           conv_w1, conv_b1, conv_w2, conv_b2, conv_w3, conv_b3,
           conv_w4, conv_b4, conv_w5, conv_b5,
           fc1_w, fc1_b, fc2_w, fc2_b):
    global LAST_EXEC_NS, LAST_RESULTS
    from concourse.bass_utils import run_bass_kernel_spmd

    conv_ws = [conv_w1, conv_w2, conv_w3, conv_w4, conv_w5]
    conv_bs = [conv_b1, conv_b2, conv_b3, conv_b4, conv_b5]
    nc, in_maps, assignment = prepare(
        encoder_output, lengths, conv_ws, conv_bs, fc1_w, fc1_b, fc2_w, fc2_b
    )

    trace = bool(int(os.environ.get("KERNEL_TRACE", "0")))
    if trace:
        trace = _ensure_ntff_hook()
    res = run_bass_kernel_spmd(nc, in_maps, list(range(NCORES)), trace=trace)
    LAST_RESULTS = res
    LAST_EXEC_NS = getattr(res, "exec_time_ns", None)

    out_full = np.empty((B, 1, 1), dtype=np.float32)
    for c in range(NCORES):
        oc = np.asarray(res.results[c]["out"]).reshape(NSLOT)
        for j in range(NSLOT):
            out_full[assignment[c][j], 0, 0] = oc[j]
    return out_full
